# revision 8
# baseline (speedup 1.0000x reference)
"""DeepSeek-V3 MoE layer on 8 Trainium2 NeuronCores.

Strategy (expert-parallel + shared-expert tensor-parallel):
  - 64 routed experts sharded 8-per-core; every core computes the dense
    per-expert gated MLP for all 256 tokens of its 8 experts in bf16 and
    accumulates  sum_e cw[t,e] * expert_e(x)[t]  into PSUM.
  - the shared GatedMLP is tensor-parallel on the intermediate dim
    (2048/8 = 256 rows per core) and accumulates into the same PSUM.
  - the noaux-tc sigmoid routing (gate matmul fp32 + grouped top-k) is
    replicated on every core; it is tiny and overlaps the expert GEMMs.
  - a ReduceScatter over the [1024, 256] partial output sums the 8 cores;
    each core returns its 128-row shard of out^T, the host concatenates
    and transposes.

Everything compute-heavy runs in bf16 (fp32 PSUM accumulation); the gate
matmul and all routing arithmetic are fp32 so the top-k selection matches
the fp32 reference exactly.

The down-projection accumulation is region-major: all 34 matmuls that
accumulate one [128, 256] output region (2 shared k-tiles + 8 experts x 4
k-tiles) are emitted as one contiguous start..stop group.  Interleaving
open accumulation groups that share a PSUM bank corrupts the early
contributions (observed on hardware); the way GEMM1 closes each group
before the next opens is the pattern that works.
"""

import sys

sys.path.insert(0, "/opt/trn_rl_repo")

import numpy as np
import ml_dtypes

import concourse.bacc as bacc
import concourse.mybir as mybir
import concourse.tile as tile
from concourse.bass_utils import run_bass_kernel_spmd

T = 256
H = 1024
E = 64
I = 512
SI = 2048
TOP_K = 6
N_GROUP = 8
TOPK_GROUP = 4
ROUTED_SCALE = 2.5
N_CORES = 8
E_LOC = E // N_CORES          # 8 experts per core
SI_LOC = SI // N_CORES        # 256 shared-intermediate rows per core
KH = H // 128                 # 8 k-tiles over hidden
KI = I // 128                 # 4 k-tiles over routed intermediate
KS = SI_LOC // 128            # 2 k-tiles over local shared intermediate
HT = H // 128                 # 8 output h-tiles

F32 = mybir.dt.float32
BF16 = mybir.dt.bfloat16
NEG = -1.0e9

_cached = None


def _build():
    nc = bacc.Bacc("TRN2", target_bir_lowering=False, debug=False, num_devices=N_CORES)

    xT_in = nc.declare_dram_parameter("xT", [H, T], F32, isOutput=False)
    xTb_in = nc.declare_dram_parameter("xTb", [128, KH * T], BF16, isOutput=False)
    gwT_in = nc.declare_dram_parameter("gwT", [H, E], F32, isOutput=False)
    eb_in = nc.declare_dram_parameter("ebias_b", [128, E], F32, isOutput=False)
    sel_in = nc.declare_dram_parameter("sel", [E, E_LOC], F32, isOutput=False)
    id_in = nc.declare_dram_parameter("ident", [128, 128], F32, isOutput=False)
    oneh_in = nc.declare_dram_parameter("oneh", [E_LOC, E_LOC * 128], F32, isOutput=False)
    w13_in = nc.declare_dram_parameter("w13T", [E_LOC, 128, KH * 2 * I], BF16, isOutput=False)
    # per-output-h-tile slabs: [ht, p(i-in-ki), e*KI*128 + ki*128 + hh]
    w2_in = nc.declare_dram_parameter("w2R", [HT, 128, E_LOC * KI * 128], BF16, isOutput=False)
    wsgu_in = nc.declare_dram_parameter("wsgu", [128, KH * 2 * SI_LOC], BF16, isOutput=False)
    wsd_in = nc.declare_dram_parameter("wsd", [128, KS * H], BF16, isOutput=False)
    out_p = nc.declare_dram_parameter("out", [128, T], BF16, isOutput=True)

    with tile.TileContext(nc) as tc:
        with (
            tc.tile_pool(name="sbuf", bufs=1) as sbuf,
            tc.tile_pool(name="wpool", bufs=3) as wpool,
            tc.tile_pool(name="w2pool", bufs=3) as w2pool,
            tc.tile_pool(name="spsum", bufs=2, space="PSUM") as spsum,
            tc.tile_pool(name="hpsum", bufs=2, space="PSUM") as hpsum,
            tc.tile_pool(name="opsum", bufs=1, space="PSUM") as opsum,
            tc.tile_pool(name="dram", bufs=1, space="DRAM") as dram,
        ):
            # ---- collective warm-up: tiny RS with no compute deps; pays the
            # per-execution collective bring-up + absorbs cross-core launch
            # skew while the real work happens.  Its result is written into a
            # corner of the output (before the real output DMA) so it is
            # never dead code.
            pre_sb = sbuf.tile([16, 16], BF16)
            nc.gpsimd.memset(pre_sb[:], 0.0)
            pre_in = dram.tile([16, 16], BF16)
            pre_out = dram.tile([2, 16], BF16)
            nc.gpsimd.dma_start(pre_in[:], pre_sb[:])
            nc.gpsimd.collective_compute(
                "ReduceScatter",
                mybir.AluOpType.add,
                replica_groups=[list(range(N_CORES))],
                ins=[pre_in.opt()],
                outs=[pre_out.opt()],
            )
            nc.gpsimd.dma_start(out_p[0:2, 0:16], pre_out[:])

            # ---- input loads: the early-critical tensors go first on the
            # sync ring so the bulk w13 stream does not starve them
            xT_sb = sbuf.tile([128, KH * T], F32)
            xTb = sbuf.tile([128, KH * T], BF16)
            xv = xT_in.ap().rearrange("(k p) t -> p k t", p=128)
            x3 = xT_sb[:].rearrange("p (k t) -> p k t", k=KH)
            nc.sync.dma_start(xTb[:], xTb_in[:])
            wsgu_sb = sbuf.tile([128, KH * 2 * SI_LOC], BF16)
            nc.sync.dma_start(wsgu_sb[:], wsgu_in[:])
            gwT_sb = sbuf.tile([128, KH * E], F32)
            nc.sync.dma_start(
                gwT_sb[:].rearrange("p (k e) -> p k e", k=KH),
                gwT_in.ap().rearrange("(k p) e -> p k e", p=128),
            )
            for c in range(4):
                nc.sync.dma_start(x3[:, 2 * c : 2 * c + 2, :], xv[:, 2 * c : 2 * c + 2, :])
            wsd_sb = sbuf.tile([128, KS * H], BF16)
            nc.sync.dma_start(wsd_sb[:], wsd_in[:])
            eb_sb = sbuf.tile([128, E], F32)
            nc.scalar.dma_start(eb_sb[:], eb_in[:])
            sel_sb = sbuf.tile([E, E_LOC], F32)
            nc.scalar.dma_start(sel_sb[:], sel_in[:])
            id_sb = sbuf.tile([128, 128], F32)
            nc.scalar.dma_start(id_sb[:], id_in[:])
            oneh_sb = sbuf.tile([E_LOC, E_LOC * 128], F32)
            nc.scalar.dma_start(oneh_sb[:], oneh_in[:])

            # ---- gate matmul (fp32): logits [t, e] per 128-token tile
            logits = []
            for tt in range(2):
                lp = spsum.tile([128, E], F32, tag="small", name=f"logits{tt}")
                for k in range(KH):
                    nc.tensor.matmul(
                        lp[:],
                        xT_sb[:, k * T + tt * 128 : k * T + tt * 128 + 128],
                        gwT_sb[:, k * E : (k + 1) * E],
                        start=(k == 0),
                        stop=(k == KH - 1),
                    )
                logits.append(lp)

            # ---- shared expert gate/up GEMMs (PE keeps busy while routing
            # runs on DVE)
            su = []
            for si in range(KS):
                sp = hpsum.tile([128, 2 * T], F32, tag="h13", name=f"su{si}")
                for k in range(KH):
                    nc.tensor.matmul(
                        sp[:, 0:T],
                        wsgu_sb[:, k * 2 * SI_LOC + si * 128 : k * 2 * SI_LOC + si * 128 + 128],
                        xTb[:, k * T : (k + 1) * T],
                        start=(k == 0),
                        stop=(k == KH - 1),
                    )
                for k in range(KH):
                    nc.tensor.matmul(
                        sp[:, T : 2 * T],
                        wsgu_sb[:, k * 2 * SI_LOC + SI_LOC + si * 128 : k * 2 * SI_LOC + SI_LOC + si * 128 + 128],
                        xTb[:, k * T : (k + 1) * T],
                        start=(k == 0),
                        stop=(k == KH - 1),
                    )
                su.append(sp)

            # ---- routing (DVE + ACT, fp32) — replicated on every core
            cw_sb = sbuf.tile([128, 2 * E], F32)
            cwT_sb = sbuf.tile([E, T], F32)
            for tt in range(2):
                scores = sbuf.tile([128, E], F32, tag="scores")
                nc.scalar.activation(scores[:], logits[tt][:], mybir.ActivationFunctionType.Sigmoid)
                swb = sbuf.tile([128, E], F32, tag="swb")
                nc.vector.tensor_add(swb[:], scores[:], eb_sb[:])
                swb3 = swb[:].rearrange("p (g j) -> p g j", g=N_GROUP)
                m1 = sbuf.tile([128, N_GROUP], F32, tag="m1")
                nc.vector.reduce_max(m1[:], swb3, axis=mybir.AxisListType.X)
                eqt = sbuf.tile([128, E], F32, tag="eqt")
                nc.vector.tensor_tensor(
                    eqt[:].rearrange("p (g j) -> p g j", g=N_GROUP),
                    swb3,
                    m1[:].to_broadcast((128, N_GROUP, E // N_GROUP)),
                    op=mybir.AluOpType.is_equal,
                )
                swb2 = sbuf.tile([128, E], F32, tag="swb2")
                nc.vector.scalar_tensor_tensor(
                    swb2[:], eqt[:], NEG, swb[:],
                    op0=mybir.AluOpType.mult, op1=mybir.AluOpType.add,
                )
                m2 = sbuf.tile([128, N_GROUP], F32, tag="m2")
                nc.vector.reduce_max(
                    m2[:], swb2[:].rearrange("p (g j) -> p g j", g=N_GROUP),
                    axis=mybir.AxisListType.X,
                )
                gsum = sbuf.tile([128, N_GROUP], F32, tag="gsum")
                nc.vector.tensor_add(gsum[:], m1[:], m2[:])
                gmask = sbuf.tile([128, N_GROUP], F32, tag="gmask")
                nc.vector.memset(gmask[:], 0.0)
                for _ in range(TOPK_GROUP):
                    gm = sbuf.tile([128, 1], F32, tag="gm")
                    nc.vector.reduce_max(gm[:], gsum[:], axis=mybir.AxisListType.X)
                    geq = sbuf.tile([128, N_GROUP], F32, tag="geq")
                    nc.vector.tensor_scalar(geq[:], gsum[:], gm[:], None, op0=mybir.AluOpType.is_equal)
                    nc.vector.tensor_add(gmask[:], gmask[:], geq[:])
                    nc.vector.scalar_tensor_tensor(
                        gsum[:], geq[:], NEG, gsum[:],
                        op0=mybir.AluOpType.mult, op1=mybir.AluOpType.add,
                    )
                swbm = sbuf.tile([128, E], F32, tag="swbm")
                nc.vector.tensor_tensor(
                    swbm[:].rearrange("p (g j) -> p g j", g=N_GROUP),
                    swb3,
                    gmask[:].to_broadcast((128, N_GROUP, E // N_GROUP)),
                    op=mybir.AluOpType.mult,
                )
                nmask = sbuf.tile([128, E], F32, tag="nmask")
                nc.vector.memset(nmask[:], 0.0)
                for _ in range(TOP_K):
                    em = sbuf.tile([128, 1], F32, tag="em")
                    nc.vector.reduce_max(em[:], swbm[:], axis=mybir.AxisListType.X)
                    eeq = sbuf.tile([128, E], F32, tag="eeq")
                    nc.vector.tensor_scalar(eeq[:], swbm[:], em[:], None, op0=mybir.AluOpType.is_equal)
                    nc.vector.tensor_add(nmask[:], nmask[:], eeq[:])
                    nc.vector.scalar_tensor_tensor(
                        swbm[:], eeq[:], NEG, swbm[:],
                        op0=mybir.AluOpType.mult, op1=mybir.AluOpType.add,
                    )
                s_sb = sbuf.tile([128, E], F32, tag="s_sb")
                nc.vector.tensor_mul(s_sb[:], scores[:], nmask[:])
                denom = sbuf.tile([128, 1], F32, tag="denom")
                nc.vector.reduce_sum(denom[:], s_sb[:], axis=mybir.AxisListType.X)
                dr = sbuf.tile([128, 1], F32, tag="dr")
                nc.vector.reciprocal(dr[:], denom[:])
                nc.vector.tensor_scalar(
                    cw_sb[:, tt * E : (tt + 1) * E], s_sb[:], dr[:], ROUTED_SCALE,
                    op0=mybir.AluOpType.mult, op1=mybir.AluOpType.mult,
                )

            # ---- shared expert activation (ACT + DVE)
            acts_sh = sbuf.tile([128, KS * T], BF16)
            for si in range(KS):
                ssl = sbuf.tile([128, T], BF16, tag="ssl")
                nc.scalar.activation(ssl[:], su[si][:, 0:T], mybir.ActivationFunctionType.Silu)
                nc.vector.tensor_mul(acts_sh[:, si * T : (si + 1) * T], ssl[:], su[si][:, T : 2 * T])

            # ---- per-expert combine weights: cb[j] = broadcast of
            # cw[:, core*8+j] across all 128 partitions
            for tt in range(2):
                ctp = spsum.tile([E, 128], F32, tag="small", name=f"ctp{tt}")
                nc.tensor.transpose(ctp[:], cw_sb[:, tt * E : (tt + 1) * E], id_sb[:])
                nc.vector.tensor_copy(cwT_sb[:, tt * 128 : (tt + 1) * 128], ctp[:])
            cwl_ps = spsum.tile([E_LOC, T], F32, tag="small")
            nc.tensor.matmul(cwl_ps[:], sel_sb[:], cwT_sb[:], start=True, stop=True)
            cwl_sb = sbuf.tile([E_LOC, T], F32)
            nc.vector.tensor_copy(cwl_sb[:], cwl_ps[:])
            cb_sb = sbuf.tile([128, E_LOC * T], BF16)
            for j in range(E_LOC):
                cbp = spsum.tile([128, T], F32, tag="small", name=f"cbp{j}")
                nc.tensor.matmul(
                    cbp[:], oneh_sb[:, j * 128 : (j + 1) * 128], cwl_sb[:],
                    start=True, stop=True,
                )
                nc.vector.tensor_copy(cb_sb[:, j * T : (j + 1) * T], cbp[:])

            # ---- routed experts: GEMM1 + activation, all 8 acts kept in SBUF
            act_sbs = []
            for e in range(E_LOC):
                w13_sb = wpool.tile([128, KH * 2 * I], BF16, tag="w13", name=f"w13_{e}")
                nc.sync.dma_start(w13_sb[:, 0 : 4 * 2 * I], w13_in[e, :, 0 : 4 * 2 * I])
                nc.sync.dma_start(w13_sb[:, 4 * 2 * I :], w13_in[e, :, 4 * 2 * I :])
                act_sb = sbuf.tile([128, KI * T], BF16, tag=f"act{e}", name=f"act{e}")
                act_sbs.append(act_sb)
                for i in range(KI):
                    hp = hpsum.tile([128, 2 * T], F32, tag="h13", name=f"h13_{e}_{i}")
                    for k in range(KH):
                        nc.tensor.matmul(
                            hp[:, 0:T],
                            w13_sb[:, k * 2 * I + i * 128 : k * 2 * I + i * 128 + 128],
                            xTb[:, k * T : (k + 1) * T],
                            start=(k == 0),
                            stop=(k == KH - 1),
                        )
                    for k in range(KH):
                        nc.tensor.matmul(
                            hp[:, T : 2 * T],
                            w13_sb[:, k * 2 * I + I + i * 128 : k * 2 * I + I + i * 128 + 128],
                            xTb[:, k * T : (k + 1) * T],
                            start=(k == 0),
                            stop=(k == KH - 1),
                        )
                    sl = sbuf.tile([128, T], BF16, tag="sl")
                    nc.scalar.activation(sl[:], hp[:, 0:T], mybir.ActivationFunctionType.Silu)
                    h3s = sbuf.tile([128, T], BF16, tag="h3s")
                    nc.vector.tensor_mul(h3s[:], hp[:, T : 2 * T], cb_sb[:, e * T : (e + 1) * T])
                    nc.vector.tensor_mul(act_sb[:, i * T : (i + 1) * T], sl[:], h3s[:])

            # ---- down-projections, region-major: one closed accumulation
            # group per [128, 256] output region
            out_ps = [opsum.tile([128, 2 * T], F32, tag=f"out{j}", name=f"out{j}") for j in range(4)]
            outf = sbuf.tile([128, HT * T], BF16)
            rs_in0 = dram.tile([H // 2, T], BF16)
            rs_in1 = dram.tile([H // 2, T], BF16)
            rs_out0 = dram.tile([64, T], BF16)
            rs_out1 = dram.tile([64, T], BF16)
            rvs = [r[:].rearrange("(j p) t -> p j t", p=128) for r in (rs_in0, rs_in1)]

            for ht in range(HT):
                reg = out_ps[ht // 2][:, (ht % 2) * T : (ht % 2) * T + T]
                w2s = w2pool.tile([128, E_LOC * KI * 128], BF16, tag="w2s", name=f"w2s{ht}")
                nc.sync.dma_start(w2s[:], w2_in[ht, :, :])
                for ks in range(KS):
                    nc.tensor.matmul(
                        reg,
                        wsd_sb[:, ks * H + ht * 128 : ks * H + ht * 128 + 128],
                        acts_sh[:, ks * T : (ks + 1) * T],
                        start=(ks == 0),
                        stop=False,
                    )
                for e in range(E_LOC):
                    for ki in range(KI):
                        nc.tensor.matmul(
                            reg,
                            w2s[:, (e * KI + ki) * 128 : (e * KI + ki) * 128 + 128],
                            act_sbs[e][:, ki * T : (ki + 1) * T],
                            start=False,
                            stop=(e == E_LOC - 1 and ki == KI - 1),
                        )
                if ht % 2 == 1:
                    j = ht // 2
                    nc.vector.tensor_copy(outf[:, j * 2 * T : (j + 1) * 2 * T], out_ps[j][:])
                    nc.sync.dma_start(
                        rvs[j // 2][:, 2 * (j % 2) : 2 * (j % 2) + 2, :],
                        outf[:, j * 2 * T : (j + 1) * 2 * T].rearrange("p (a t) -> p a t", a=2),
                    )
                    if j == 1:
                        # first half of the output is complete: reduce it while
                        # the second half is still computing
                        nc.gpsimd.collective_compute(
                            "ReduceScatter",
                            mybir.AluOpType.add,
                            replica_groups=[list(range(N_CORES))],
                            ins=[rs_in0.opt()],
                            outs=[rs_out0.opt()],
                        )

            nc.gpsimd.collective_compute(
                "ReduceScatter",
                mybir.AluOpType.add,
                replica_groups=[list(range(N_CORES))],
                ins=[rs_in1.opt()],
                outs=[rs_out1.opt()],
            )
            nc.sync.dma_start(out_p[0:64, :], rs_out0[:])
            nc.sync.dma_start(out_p[64:128, :], rs_out1[:])

    nc.finalize()
    return nc


def _prep_inputs(inputs):
    bf = ml_dtypes.bfloat16
    x = np.asarray(inputs["hidden_states"], np.float32)
    gate_w = np.asarray(inputs["gate_w"], np.float32)
    e_bias = np.asarray(inputs["e_bias"], np.float32)
    w1 = np.asarray(inputs["w1"], np.float32)
    w3 = np.asarray(inputs["w3"], np.float32)
    w2 = np.asarray(inputs["w2"], np.float32)
    ws_gate = np.asarray(inputs["ws_gate"], np.float32)
    ws_up = np.asarray(inputs["ws_up"], np.float32)
    ws_down = np.asarray(inputs["ws_down"], np.float32)

    xT = np.ascontiguousarray(x.T)
    xTb = np.ascontiguousarray(x.T.reshape(KH, 128, T).transpose(1, 0, 2).reshape(128, KH * T)).astype(bf)
    gwT = np.ascontiguousarray(gate_w.T)
    ebb = np.broadcast_to(e_bias[None, :], (128, E)).copy()
    ident = np.eye(128, dtype=np.float32)
    oneh = np.zeros((E_LOC, E_LOC * 128), np.float32)
    for j in range(E_LOC):
        oneh[j, j * 128 : (j + 1) * 128] = 1.0

    # routed up/gate weights: [E, k, p, ...] -> [E, p, k*...]
    w1t = w1.transpose(0, 2, 1).reshape(E, KH, 128, I)
    w3t = w3.transpose(0, 2, 1).reshape(E, KH, 128, I)
    w13 = np.concatenate([w1t, w3t], axis=-1)          # [E, KH, 128, 2I]
    w13 = w13.transpose(0, 2, 1, 3).reshape(E, 128, KH * 2 * I).astype(bf)
    # routed down weights, packed per output h-tile:
    # w2R[c][ht, p, (e*KI + ki)*128 + hh] = w2[8c+e][ht*128+hh, ki*128+p]
    w2t = w2.transpose(0, 2, 1).reshape(E, KI, 128, HT, 128)  # [e, ki, p, ht, hh]

    in_maps = []
    for c in range(N_CORES):
        sel = np.zeros((E, E_LOC), np.float32)
        for j in range(E_LOC):
            sel[c * E_LOC + j, j] = 1.0
        wsg = ws_gate[c * SI_LOC : (c + 1) * SI_LOC, :].T.reshape(KH, 128, SI_LOC)
        wsu = ws_up[c * SI_LOC : (c + 1) * SI_LOC, :].T.reshape(KH, 128, SI_LOC)
        wsgu = np.concatenate([wsg, wsu], axis=-1).transpose(1, 0, 2).reshape(128, KH * 2 * SI_LOC).astype(bf)
        wsd = ws_down[:, c * SI_LOC : (c + 1) * SI_LOC].T.reshape(KS, 128, H)
        wsd = wsd.transpose(1, 0, 2).reshape(128, KS * H).astype(bf)
        w2r = w2t[c * E_LOC : (c + 1) * E_LOC].transpose(3, 2, 0, 1, 4)  # [ht, p, e, ki, hh]
        w2r = np.ascontiguousarray(w2r.reshape(HT, 128, E_LOC * KI * 128)).astype(bf)
        in_maps.append(
            {
                "xT": xT,
                "xTb": xTb,
                "gwT": gwT,
                "ebias_b": ebb,
                "sel": sel,
                "ident": ident,
                "oneh": oneh,
                "w13T": np.ascontiguousarray(w13[c * E_LOC : (c + 1) * E_LOC]),
                "w2R": w2r,
                "wsgu": wsgu,
                "wsd": wsd,
            }
        )
    return in_maps


last_result = None


def kernel(**inputs):
    global _cached, last_result
    trace = bool(inputs.pop("_trace", False))
    if _cached is None:
        _cached = _build()
    nc = _cached
    in_maps = _prep_inputs(inputs)
    res = run_bass_kernel_spmd(nc, in_maps, core_ids=list(range(N_CORES)), trace=trace)
    last_result = res
    top = np.concatenate([res.results[c]["out"][0:64] for c in range(N_CORES)], axis=0)
    bot = np.concatenate([res.results[c]["out"][64:128] for c in range(N_CORES)], axis=0)
    outT = np.concatenate([top, bot], axis=0).astype(np.float32)
    return np.ascontiguousarray(outT.T)


# revision 9
# speedup vs baseline: 1.0420x; 1.0420x over previous
"""DeepSeek-V3 MoE layer on 8 Trainium2 NeuronCores.

Strategy (expert-parallel + shared-expert tensor-parallel):
  - 64 routed experts sharded 8-per-core; every core computes the dense
    per-expert gated MLP for all 256 tokens of its 8 experts in bf16 and
    accumulates  sum_e cw[t,e] * expert_e(x)[t]  into PSUM.
  - the shared GatedMLP is tensor-parallel on the intermediate dim
    (2048/8 = 256 rows per core) and accumulates into the same PSUM.
  - the noaux-tc sigmoid routing (gate matmul fp32 + grouped top-k) is
    replicated on every core; it is tiny and overlaps the expert GEMMs.
  - a ReduceScatter over the [1024, 256] partial output sums the 8 cores;
    each core returns its 128-row shard of out^T, the host concatenates
    and transposes.

Everything compute-heavy runs in bf16 (fp32 PSUM accumulation); the gate
matmul and all routing arithmetic are fp32 so the top-k selection matches
the fp32 reference exactly.

The down-projection accumulation is region-major: all 34 matmuls that
accumulate one [128, 256] output region (2 shared k-tiles + 8 experts x 4
k-tiles) are emitted as one contiguous start..stop group.  Interleaving
open accumulation groups that share a PSUM bank corrupts the early
contributions (observed on hardware); the way GEMM1 closes each group
before the next opens is the pattern that works.
"""

import sys

sys.path.insert(0, "/opt/trn_rl_repo")

import numpy as np
import ml_dtypes

import concourse.bacc as bacc
import concourse.mybir as mybir
import concourse.tile as tile
from concourse.bass_utils import run_bass_kernel_spmd

T = 256
H = 1024
E = 64
I = 512
SI = 2048
TOP_K = 6
N_GROUP = 8
TOPK_GROUP = 4
ROUTED_SCALE = 2.5
N_CORES = 8
E_LOC = E // N_CORES          # 8 experts per core
SI_LOC = SI // N_CORES        # 256 shared-intermediate rows per core
KH = H // 128                 # 8 k-tiles over hidden
KI = I // 128                 # 4 k-tiles over routed intermediate
KS = SI_LOC // 128            # 2 k-tiles over local shared intermediate
HT = H // 128                 # 8 output h-tiles

F32 = mybir.dt.float32
BF16 = mybir.dt.bfloat16
NEG = -1.0e9

_cached = None


def _build():
    nc = bacc.Bacc("TRN2", target_bir_lowering=False, debug=False, num_devices=N_CORES)

    xT_in = nc.declare_dram_parameter("xT", [H, T], F32, isOutput=False)
    xTb_in = nc.declare_dram_parameter("xTb", [128, KH * T], BF16, isOutput=False)
    gwT_in = nc.declare_dram_parameter("gwT", [H, E], F32, isOutput=False)
    eb_in = nc.declare_dram_parameter("ebias_b", [128, E], F32, isOutput=False)
    sel_in = nc.declare_dram_parameter("sel", [E, E_LOC], F32, isOutput=False)
    id_in = nc.declare_dram_parameter("ident", [128, 128], F32, isOutput=False)
    oneh_in = nc.declare_dram_parameter("oneh", [E_LOC, E_LOC * 128], F32, isOutput=False)
    w13_in = nc.declare_dram_parameter("w13T", [E_LOC, 128, KH * 2 * I], BF16, isOutput=False)
    # per-output-h-tile slabs: [ht, p(i-in-ki), e*KI*128 + ki*128 + hh]
    w2_in = nc.declare_dram_parameter("w2R", [HT, 128, E_LOC * KI * 128], BF16, isOutput=False)
    wsgu_in = nc.declare_dram_parameter("wsgu", [128, KH * 2 * SI_LOC], BF16, isOutput=False)
    wsd_in = nc.declare_dram_parameter("wsd", [128, KS * H], BF16, isOutput=False)
    out_p = nc.declare_dram_parameter("out", [128, T], BF16, isOutput=True)

    with tile.TileContext(nc) as tc:
        with (
            tc.tile_pool(name="sbuf", bufs=1) as sbuf,
            tc.tile_pool(name="wpool", bufs=3) as wpool,
            tc.tile_pool(name="w2pool", bufs=3) as w2pool,
            tc.tile_pool(name="spsum", bufs=2, space="PSUM") as spsum,
            tc.tile_pool(name="hpsum", bufs=2, space="PSUM") as hpsum,
            tc.tile_pool(name="opsum", bufs=1, space="PSUM") as opsum,
            tc.tile_pool(name="dram", bufs=1, space="DRAM") as dram,
        ):
            # ---- collective warm-up: tiny RS with no compute deps; pays the
            # per-execution collective bring-up + absorbs cross-core launch
            # skew while the real work happens.  Its result is written into a
            # corner of the output (before the real output DMA) so it is
            # never dead code.
            pre_sb = sbuf.tile([16, 16], BF16)
            nc.gpsimd.memset(pre_sb[:], 0.0)
            pre_in = dram.tile([16, 16], BF16)
            pre_out = dram.tile([2, 16], BF16)
            nc.gpsimd.dma_start(pre_in[:], pre_sb[:])
            nc.gpsimd.collective_compute(
                "ReduceScatter",
                mybir.AluOpType.add,
                replica_groups=[list(range(N_CORES))],
                ins=[pre_in.opt()],
                outs=[pre_out.opt()],
            )
            nc.gpsimd.dma_start(out_p[0:2, 0:16], pre_out[:])

            # ---- input loads: the early-critical tensors go first on the
            # sync ring so the bulk w13 stream does not starve them
            xT_sb = sbuf.tile([128, KH * T], F32)
            xTb = sbuf.tile([128, KH * T], BF16)
            xv = xT_in.ap().rearrange("(k p) t -> p k t", p=128)
            x3 = xT_sb[:].rearrange("p (k t) -> p k t", k=KH)
            nc.sync.dma_start(xTb[:], xTb_in[:])
            wsgu_sb = sbuf.tile([128, KH * 2 * SI_LOC], BF16)
            nc.sync.dma_start(wsgu_sb[:], wsgu_in[:])
            gwT_sb = sbuf.tile([128, KH * E], F32)
            nc.sync.dma_start(
                gwT_sb[:].rearrange("p (k e) -> p k e", k=KH),
                gwT_in.ap().rearrange("(k p) e -> p k e", p=128),
            )
            for c in range(4):
                nc.sync.dma_start(x3[:, 2 * c : 2 * c + 2, :], xv[:, 2 * c : 2 * c + 2, :])
            wsd_sb = sbuf.tile([128, KS * H], BF16)
            nc.sync.dma_start(wsd_sb[:], wsd_in[:])
            eb_sb = sbuf.tile([128, E], F32)
            nc.scalar.dma_start(eb_sb[:], eb_in[:])
            sel_sb = sbuf.tile([E, E_LOC], F32)
            nc.scalar.dma_start(sel_sb[:], sel_in[:])
            id_sb = sbuf.tile([128, 128], F32)
            nc.scalar.dma_start(id_sb[:], id_in[:])
            oneh_sb = sbuf.tile([E_LOC, E_LOC * 128], F32)
            nc.scalar.dma_start(oneh_sb[:], oneh_in[:])

            # ---- gate matmul (fp32): logits [t, e] per 128-token tile
            logits = []
            for tt in range(2):
                lp = spsum.tile([128, E], F32, tag="small", name=f"logits{tt}")
                for k in range(KH):
                    nc.tensor.matmul(
                        lp[:],
                        xT_sb[:, k * T + tt * 128 : k * T + tt * 128 + 128],
                        gwT_sb[:, k * E : (k + 1) * E],
                        start=(k == 0),
                        stop=(k == KH - 1),
                    )
                logits.append(lp)

            # ---- shared expert gate/up GEMMs (PE keeps busy while routing
            # runs on DVE)
            su = []
            for si in range(KS):
                sp = hpsum.tile([128, 2 * T], F32, tag="h13", name=f"su{si}")
                for k in range(KH):
                    nc.tensor.matmul(
                        sp[:, 0:T],
                        wsgu_sb[:, k * 2 * SI_LOC + si * 128 : k * 2 * SI_LOC + si * 128 + 128],
                        xTb[:, k * T : (k + 1) * T],
                        start=(k == 0),
                        stop=(k == KH - 1),
                    )
                for k in range(KH):
                    nc.tensor.matmul(
                        sp[:, T : 2 * T],
                        wsgu_sb[:, k * 2 * SI_LOC + SI_LOC + si * 128 : k * 2 * SI_LOC + SI_LOC + si * 128 + 128],
                        xTb[:, k * T : (k + 1) * T],
                        start=(k == 0),
                        stop=(k == KH - 1),
                    )
                su.append(sp)

            # ---- routing (DVE + ACT, fp32) — replicated on every core
            cw_sb = sbuf.tile([128, 2 * E], F32)
            cwT_sb = sbuf.tile([E, T], F32)
            for tt in range(2):
                scores = sbuf.tile([128, E], F32, tag="scores")
                nc.scalar.activation(scores[:], logits[tt][:], mybir.ActivationFunctionType.Sigmoid)
                swb = sbuf.tile([128, E], F32, tag="swb")
                nc.vector.tensor_add(swb[:], scores[:], eb_sb[:])
                swb3 = swb[:].rearrange("p (g j) -> p g j", g=N_GROUP)
                m1 = sbuf.tile([128, N_GROUP], F32, tag="m1")
                nc.vector.reduce_max(m1[:], swb3, axis=mybir.AxisListType.X)
                eqt = sbuf.tile([128, E], F32, tag="eqt")
                nc.vector.tensor_tensor(
                    eqt[:].rearrange("p (g j) -> p g j", g=N_GROUP),
                    swb3,
                    m1[:].to_broadcast((128, N_GROUP, E // N_GROUP)),
                    op=mybir.AluOpType.is_equal,
                )
                swb2 = sbuf.tile([128, E], F32, tag="swb2")
                nc.vector.scalar_tensor_tensor(
                    swb2[:], eqt[:], NEG, swb[:],
                    op0=mybir.AluOpType.mult, op1=mybir.AluOpType.add,
                )
                m2 = sbuf.tile([128, N_GROUP], F32, tag="m2")
                nc.vector.reduce_max(
                    m2[:], swb2[:].rearrange("p (g j) -> p g j", g=N_GROUP),
                    axis=mybir.AxisListType.X,
                )
                gsum = sbuf.tile([128, N_GROUP], F32, tag="gsum")
                nc.vector.tensor_add(gsum[:], m1[:], m2[:])
                gmask = sbuf.tile([128, N_GROUP], F32, tag="gmask")
                nc.vector.memset(gmask[:], 0.0)
                for _ in range(TOPK_GROUP):
                    gm = sbuf.tile([128, 1], F32, tag="gm")
                    nc.vector.reduce_max(gm[:], gsum[:], axis=mybir.AxisListType.X)
                    geq = sbuf.tile([128, N_GROUP], F32, tag="geq")
                    nc.vector.tensor_scalar(geq[:], gsum[:], gm[:], None, op0=mybir.AluOpType.is_equal)
                    nc.vector.tensor_add(gmask[:], gmask[:], geq[:])
                    nc.vector.scalar_tensor_tensor(
                        gsum[:], geq[:], NEG, gsum[:],
                        op0=mybir.AluOpType.mult, op1=mybir.AluOpType.add,
                    )
                swbm = sbuf.tile([128, E], F32, tag="swbm")
                nc.vector.tensor_tensor(
                    swbm[:].rearrange("p (g j) -> p g j", g=N_GROUP),
                    swb3,
                    gmask[:].to_broadcast((128, N_GROUP, E // N_GROUP)),
                    op=mybir.AluOpType.mult,
                )
                nmask = sbuf.tile([128, E], F32, tag="nmask")
                nc.vector.memset(nmask[:], 0.0)
                for _ in range(TOP_K):
                    em = sbuf.tile([128, 1], F32, tag="em")
                    nc.vector.reduce_max(em[:], swbm[:], axis=mybir.AxisListType.X)
                    eeq = sbuf.tile([128, E], F32, tag="eeq")
                    nc.vector.tensor_scalar(eeq[:], swbm[:], em[:], None, op0=mybir.AluOpType.is_equal)
                    nc.vector.tensor_add(nmask[:], nmask[:], eeq[:])
                    nc.vector.scalar_tensor_tensor(
                        swbm[:], eeq[:], NEG, swbm[:],
                        op0=mybir.AluOpType.mult, op1=mybir.AluOpType.add,
                    )
                s_sb = sbuf.tile([128, E], F32, tag="s_sb")
                nc.vector.tensor_mul(s_sb[:], scores[:], nmask[:])
                denom = sbuf.tile([128, 1], F32, tag="denom")
                nc.vector.reduce_sum(denom[:], s_sb[:], axis=mybir.AxisListType.X)
                dr = sbuf.tile([128, 1], F32, tag="dr")
                nc.vector.reciprocal(dr[:], denom[:])
                nc.vector.tensor_scalar(
                    cw_sb[:, tt * E : (tt + 1) * E], s_sb[:], dr[:], ROUTED_SCALE,
                    op0=mybir.AluOpType.mult, op1=mybir.AluOpType.mult,
                )

            # ---- shared expert activation (ACT + DVE)
            acts_sh = sbuf.tile([128, KS * T], BF16)
            for si in range(KS):
                ssl = sbuf.tile([128, T], BF16, tag="ssl")
                nc.scalar.activation(ssl[:], su[si][:, 0:T], mybir.ActivationFunctionType.Silu)
                nc.vector.tensor_mul(acts_sh[:, si * T : (si + 1) * T], ssl[:], su[si][:, T : 2 * T])

            # ---- per-expert combine weights: cb[j] = broadcast of
            # cw[:, core*8+j] across all 128 partitions
            for tt in range(2):
                ctp = spsum.tile([E, 128], F32, tag="small", name=f"ctp{tt}")
                nc.tensor.transpose(ctp[:], cw_sb[:, tt * E : (tt + 1) * E], id_sb[:])
                nc.vector.tensor_copy(cwT_sb[:, tt * 128 : (tt + 1) * 128], ctp[:])
            cwl_ps = spsum.tile([E_LOC, T], F32, tag="small")
            nc.tensor.matmul(cwl_ps[:], sel_sb[:], cwT_sb[:], start=True, stop=True)
            cwl_sb = sbuf.tile([E_LOC, T], F32)
            nc.vector.tensor_copy(cwl_sb[:], cwl_ps[:])
            cb_sb = sbuf.tile([128, E_LOC * T], BF16)
            for j in range(E_LOC):
                cbp = spsum.tile([128, T], F32, tag="small", name=f"cbp{j}")
                nc.tensor.matmul(
                    cbp[:], oneh_sb[:, j * 128 : (j + 1) * 128], cwl_sb[:],
                    start=True, stop=True,
                )
                nc.vector.tensor_copy(cb_sb[:, j * T : (j + 1) * T], cbp[:])

            # ---- routed experts: GEMM1 + activation, all 8 acts kept in SBUF
            act_sbs = []
            for e in range(E_LOC):
                w13_sb = wpool.tile([128, KH * 2 * I], BF16, tag="w13", name=f"w13_{e}")
                nc.sync.dma_start(w13_sb[:, 0 : 4 * 2 * I], w13_in[e, :, 0 : 4 * 2 * I])
                nc.sync.dma_start(w13_sb[:, 4 * 2 * I :], w13_in[e, :, 4 * 2 * I :])
                act_sb = sbuf.tile([128, KI * T], BF16, tag=f"act{e}", name=f"act{e}")
                act_sbs.append(act_sb)
                for i in range(KI):
                    hp = hpsum.tile([128, 2 * T], F32, tag="h13", name=f"h13_{e}_{i}")
                    for k in range(KH):
                        nc.tensor.matmul(
                            hp[:, 0:T],
                            w13_sb[:, k * 2 * I + i * 128 : k * 2 * I + i * 128 + 128],
                            xTb[:, k * T : (k + 1) * T],
                            start=(k == 0),
                            stop=(k == KH - 1),
                        )
                    for k in range(KH):
                        nc.tensor.matmul(
                            hp[:, T : 2 * T],
                            w13_sb[:, k * 2 * I + I + i * 128 : k * 2 * I + I + i * 128 + 128],
                            xTb[:, k * T : (k + 1) * T],
                            start=(k == 0),
                            stop=(k == KH - 1),
                        )
                    sl = sbuf.tile([128, T], BF16, tag="sl")
                    nc.scalar.activation(sl[:], hp[:, 0:T], mybir.ActivationFunctionType.Silu)
                    h3s = sbuf.tile([128, T], BF16, tag="h3s")
                    nc.vector.tensor_mul(h3s[:], hp[:, T : 2 * T], cb_sb[:, e * T : (e + 1) * T])
                    nc.vector.tensor_mul(act_sb[:, i * T : (i + 1) * T], sl[:], h3s[:])

            # ---- down-projections, region-major: one closed accumulation
            # group per [128, 256] output region
            out_ps = [opsum.tile([128, 2 * T], F32, tag=f"out{j}", name=f"out{j}") for j in range(4)]
            outf = sbuf.tile([128, HT * T], BF16)
            rs_in0 = dram.tile([H // 2, T], BF16)
            rs_in1 = dram.tile([H // 2, T], BF16)
            rs_out0 = dram.tile([64, T], BF16)
            rs_out1 = dram.tile([64, T], BF16)
            rvs = [r[:].rearrange("(j p) t -> p j t", p=128) for r in (rs_in0, rs_in1)]

            for ht in range(HT):
                reg = out_ps[ht // 2][:, (ht % 2) * T : (ht % 2) * T + T]
                w2s = w2pool.tile([128, E_LOC * KI * 128], BF16, tag="w2s", name=f"w2s{ht}")
                nc.sync.dma_start(w2s[:], w2_in[ht, :, :])
                for ks in range(KS):
                    nc.tensor.matmul(
                        reg,
                        wsd_sb[:, ks * H + ht * 128 : ks * H + ht * 128 + 128],
                        acts_sh[:, ks * T : (ks + 1) * T],
                        start=(ks == 0),
                        stop=False,
                    )
                for e in range(E_LOC):
                    for ki in range(KI):
                        nc.tensor.matmul(
                            reg,
                            w2s[:, (e * KI + ki) * 128 : (e * KI + ki) * 128 + 128],
                            act_sbs[e][:, ki * T : (ki + 1) * T],
                            start=False,
                            stop=(e == E_LOC - 1 and ki == KI - 1),
                        )
                if ht % 2 == 1:
                    j = ht // 2
                    nc.vector.tensor_copy(outf[:, j * 2 * T : (j + 1) * 2 * T], out_ps[j][:])
                    nc.sync.dma_start(
                        rvs[j // 2][:, 2 * (j % 2) : 2 * (j % 2) + 2, :],
                        outf[:, j * 2 * T : (j + 1) * 2 * T].rearrange("p (a t) -> p a t", a=2),
                    )
            nc.gpsimd.collective_compute(
                "ReduceScatter",
                mybir.AluOpType.add,
                replica_groups=[list(range(N_CORES))],
                ins=[rs_in0.opt()],
                outs=[rs_out0.opt()],
            )
            nc.gpsimd.collective_compute(
                "ReduceScatter",
                mybir.AluOpType.add,
                replica_groups=[list(range(N_CORES))],
                ins=[rs_in1.opt()],
                outs=[rs_out1.opt()],
            )
            nc.sync.dma_start(out_p[0:64, :], rs_out0[:])
            nc.sync.dma_start(out_p[64:128, :], rs_out1[:])

    nc.finalize()
    return nc


def _prep_inputs(inputs):
    bf = ml_dtypes.bfloat16
    x = np.asarray(inputs["hidden_states"], np.float32)
    gate_w = np.asarray(inputs["gate_w"], np.float32)
    e_bias = np.asarray(inputs["e_bias"], np.float32)
    w1 = np.asarray(inputs["w1"], np.float32)
    w3 = np.asarray(inputs["w3"], np.float32)
    w2 = np.asarray(inputs["w2"], np.float32)
    ws_gate = np.asarray(inputs["ws_gate"], np.float32)
    ws_up = np.asarray(inputs["ws_up"], np.float32)
    ws_down = np.asarray(inputs["ws_down"], np.float32)

    xT = np.ascontiguousarray(x.T)
    xTb = np.ascontiguousarray(x.T.reshape(KH, 128, T).transpose(1, 0, 2).reshape(128, KH * T)).astype(bf)
    gwT = np.ascontiguousarray(gate_w.T)
    ebb = np.broadcast_to(e_bias[None, :], (128, E)).copy()
    ident = np.eye(128, dtype=np.float32)
    oneh = np.zeros((E_LOC, E_LOC * 128), np.float32)
    for j in range(E_LOC):
        oneh[j, j * 128 : (j + 1) * 128] = 1.0

    # routed up/gate weights: [E, k, p, ...] -> [E, p, k*...]
    w1t = w1.transpose(0, 2, 1).reshape(E, KH, 128, I)
    w3t = w3.transpose(0, 2, 1).reshape(E, KH, 128, I)
    w13 = np.concatenate([w1t, w3t], axis=-1)          # [E, KH, 128, 2I]
    w13 = w13.transpose(0, 2, 1, 3).reshape(E, 128, KH * 2 * I).astype(bf)
    # routed down weights, packed per output h-tile:
    # w2R[c][ht, p, (e*KI + ki)*128 + hh] = w2[8c+e][ht*128+hh, ki*128+p]
    w2t = w2.transpose(0, 2, 1).reshape(E, KI, 128, HT, 128)  # [e, ki, p, ht, hh]

    in_maps = []
    for c in range(N_CORES):
        sel = np.zeros((E, E_LOC), np.float32)
        for j in range(E_LOC):
            sel[c * E_LOC + j, j] = 1.0
        wsg = ws_gate[c * SI_LOC : (c + 1) * SI_LOC, :].T.reshape(KH, 128, SI_LOC)
        wsu = ws_up[c * SI_LOC : (c + 1) * SI_LOC, :].T.reshape(KH, 128, SI_LOC)
        wsgu = np.concatenate([wsg, wsu], axis=-1).transpose(1, 0, 2).reshape(128, KH * 2 * SI_LOC).astype(bf)
        wsd = ws_down[:, c * SI_LOC : (c + 1) * SI_LOC].T.reshape(KS, 128, H)
        wsd = wsd.transpose(1, 0, 2).reshape(128, KS * H).astype(bf)
        w2r = w2t[c * E_LOC : (c + 1) * E_LOC].transpose(3, 2, 0, 1, 4)  # [ht, p, e, ki, hh]
        w2r = np.ascontiguousarray(w2r.reshape(HT, 128, E_LOC * KI * 128)).astype(bf)
        in_maps.append(
            {
                "xT": xT,
                "xTb": xTb,
                "gwT": gwT,
                "ebias_b": ebb,
                "sel": sel,
                "ident": ident,
                "oneh": oneh,
                "w13T": np.ascontiguousarray(w13[c * E_LOC : (c + 1) * E_LOC]),
                "w2R": w2r,
                "wsgu": wsgu,
                "wsd": wsd,
            }
        )
    return in_maps


last_result = None


def kernel(**inputs):
    global _cached, last_result
    trace = bool(inputs.pop("_trace", False))
    if _cached is None:
        _cached = _build()
    nc = _cached
    in_maps = _prep_inputs(inputs)
    res = run_bass_kernel_spmd(nc, in_maps, core_ids=list(range(N_CORES)), trace=trace)
    last_result = res
    top = np.concatenate([res.results[c]["out"][0:64] for c in range(N_CORES)], axis=0)
    bot = np.concatenate([res.results[c]["out"][64:128] for c in range(N_CORES)], axis=0)
    outT = np.concatenate([top, bot], axis=0).astype(np.float32)
    return np.ascontiguousarray(outT.T)
